# revision 21
# baseline (speedup 1.0000x reference)
"""Bass/Trainium2 kernel for nn_GCL_49959059587771 (GCL JSD loss).

Math: for pair (z, g, batch), with zn/gn row-normalized:
    s_i = <zn_i, gn_self[b_i]>,  c_i = <zn_i, gn_cross[b_i]>
    d_i = softplus(-c_i) - softplus(-s_i)
    L = sqrt(sum d1^2) + sqrt(sum d2^2)
(the one-hot mask collapses each [N, G] row to one entry; ep_jsd(0) = 0.)

Device strategy (8 cores, nodes sharded, v2 "tiny-window P" scheme):
  Host normalizes z and g rows (pure preprocessing, like the one-hot build)
  and ships per core:
    - zT      [128, 2, 6656] fp8: d-major transposed normalized z chunks
    - st      [128, 2, 16, 2, 32] fp8: per-(pair, chunk) stationary of 16
              self + 16 cross normalized-g window rows (batch is sorted, so
              each 512-node chunk spans < 16 distinct g rows)
    - oh      [128, 4, 512] bf16 per pair: one-hot window masks, 4 chunks
              stacked per 32-partition block
    - sel     [128, 32] bf16: partition-reduce stationary (s rows 0:4,
              c rows 8:12 per 32-block)
  Per (pair, group of 4 chunks): 8 tiny-stationary matmuls produce
  P[32*j + w, i] = <zn_i, gwin_w> in one PSUM bank; one DVE mask op and one
  select matmul reduce it to per-chunk s/c rows packed in a per-pair
  "select bank"; one ACT Softplus(-x) + 4 DVE subs + 1 DVE square-accум
  produce the per-pair sum(d^2) partials. Host sums partials, sqrt, add.
"""

import numpy as np
from contextlib import ExitStack

import concourse.bass as bass
import concourse.bacc as bacc
import concourse.tile as tile
import concourse.mybir as mybir
from concourse.bass_utils import run_bass_kernel_spmd

N, G, D = 50000, 512, 256
NCORES = 8
RPC = N // NCORES            # 6250 nodes per core
CHUNK = 512                  # nodes per chunk
NCH = 13                     # real chunks per core (13*512 = 6656 >= 6250)
NODES = NCH * CHUNK          # 6656 padded nodes per core
NGRP = 4                     # groups of 4 chunks (last group: 1 real chunk)
WIN = 16                     # g-row window per chunk (span asserted < 16)

AF = mybir.ActivationFunctionType
ALU = mybir.AluOpType
F32 = mybir.dt.float32
BF16 = mybir.dt.bfloat16
FP8 = mybir.dt.float8e4

Z_DT = FP8                   # dtype of zT and st (PE operands)


def build(z_dt=Z_DT, debug=False):
    nc = bacc.Bacc("TRN2", target_bir_lowering=False, debug=debug)

    zT1 = nc.dram_tensor("zT1", [128, 2, NODES], z_dt, kind="ExternalInput")
    zT2 = nc.dram_tensor("zT2", [128, 2, NODES], z_dt, kind="ExternalInput")
    oh1 = nc.dram_tensor("oh1", [128, NGRP, CHUNK], z_dt,
                         kind="ExternalInput")
    oh2 = nc.dram_tensor("oh2", [128, NGRP, CHUNK], z_dt,
                         kind="ExternalInput")
    st = nc.dram_tensor("st", [128, 2, 16, 2, 2 * WIN], z_dt,
                        kind="ExternalInput")
    sel = nc.dram_tensor("sel", [128, 2, 2 * WIN], BF16,
                         kind="ExternalInput")
    acc = nc.dram_tensor("acc", [128, 2], F32, kind="ExternalOutput")

    with tile.TileContext(nc) as tc, ExitStack() as ctx:
        singles = ctx.enter_context(tc.tile_pool(name="singles", bufs=1))
        zpool = ctx.enter_context(tc.tile_pool(name="z", bufs=3))
        mpool = ctx.enter_context(tc.tile_pool(name="m", bufs=3))
        ppool = ctx.enter_context(tc.tile_pool(name="pp", bufs=3,
                                               space="PSUM"))
        spool = ctx.enter_context(tc.tile_pool(name="sp", bufs=1,
                                               space="PSUM"))

        # input DMAs are spread over three engine queues so descriptor
        # generation and the transfers themselves overlap
        st_sb = singles.tile([128, 2, 16, 2, 2 * WIN], z_dt)
        nc.sync.dma_start(st_sb[:], st[:])
        sel_sb = singles.tile([128, 2, 2 * WIN], BF16)
        nc.gpsimd.dma_start(sel_sb[:], sel[:])
        oh_sb = []
        for p, oh in enumerate((oh1, oh2)):
            t = singles.tile([128, NGRP, CHUNK], z_dt, tag=f"oh{p}")
            nc.gpsimd.dma_start(t[:], oh[:])
            oh_sb.append(t)
        # z group loads: [128, 2, 2048] per (pair, group); the last group
        # carries only its single real chunk (512)
        zg = [[None] * NGRP for _ in range(2)]
        for p, zT in enumerate((zT1, zT2)):
            eng = nc.sync if p == 0 else nc.scalar
            for g in range(NGRP):
                lo = g * 4 * CHUNK
                hi = lo + min(4, NCH - 4 * g) * CHUNK
                t = zpool.tile([128, 2, hi - lo], z_dt, tag=f"z{p}{g}")
                eng.dma_start(t[:], zT[:, :, lo:hi])
                zg[p][g] = t

        # warm-up matmuls on a zeroed tile keep the PE HAM busy while the
        # first z transfer is in flight (otherwise every burst runs cold)
        junkz = singles.tile([128, CHUNK], BF16)
        nc.vector.memset(junkz[:], 0.0)
        dummy_ps = ctx.enter_context(tc.tile_pool(name="dps", bufs=1,
                                                  space="PSUM"))
        dps = dummy_ps.tile([128, CHUNK], F32)
        for _ in range(8):
            nc.tensor.matmul(dps[:], junkz[:, 0:128], junkz[:],
                             start=True, stop=True)

        # selbank[p][e]: e=0 holds s rows, e=1 holds c rows (chunk 4g+j of
        # pair p at partition 32g + j, only rows 32g + 0:4 of each block used)
        selbank = [[spool.tile([128, CHUNK], F32, name=f"selbank{p}{e}",
                               tag=f"sb{p}{e}")
                    for e in range(2)] for p in range(2)]

        # software-pipelined main loop: P-matmuls run one group ahead of
        # the dependent (DVE-gated) select matmuls so PE never stalls.
        work = [(p, g) for p in range(2) for g in range(NGRP)]
        pbank = {}

        def p_mms(p, g):
            nj = min(4, NCH - 4 * g)
            P = ppool.tile([128, CHUNK], F32, tag="P")
            for j in range(nj):
                c = 4 * g + j
                for k in range(2):
                    nc.tensor.matmul(
                        P[32 * j:32 * j + 32, :],
                        st_sb[:, p, c, k, :],
                        zg[p][g][:, k, (c - 4 * g) * CHUNK:
                                 (c - 4 * g + 1) * CHUNK],
                        start=(k == 0), stop=(k == 1),
                        tile_position=(0, 32 * j))
            pbank[(p, g)] = (P, nj)

        def mask_select(p, g):
            P, nj = pbank.pop((p, g))
            np_ = 32 * nj
            masked = mpool.tile([128, CHUNK], BF16, tag="masked")
            nc.vector.scalar_tensor_tensor(
                out=masked[0:np_, :], in0=oh_sb[p][0:np_, g, :], scalar=1.0,
                in1=P[0:np_, :], op0=ALU.mult, op1=ALU.mult)
            for e in range(2):
                nc.tensor.matmul(
                    selbank[p][e][32 * g:32 * g + 32, :],
                    sel_sb[0:np_, e, :], masked[0:np_, :],
                    start=True, stop=True, tile_position=(0, 32 * g))

        p_mms(*work[0])
        for i, (p, g) in enumerate(work):
            if i + 1 < len(work):
                p_mms(*work[i + 1])
            mask_select(p, g)

        # epilogue: softplus(-x) = ln(1 + exp(-x)) (Exp and Ln share one
        # table set); d = sp(-c) - sp(-s); accumulate d^2. Unused rows hold
        # selbank 0 -> sp ln2 on both sides -> d = 0.
        acc_sb = singles.tile([128, 2], F32)
        tiny = singles.tile([128, 1], F32)
        for p in range(2):
            spl = []
            exs = []
            for e in range(2):
                ex = mpool.tile([128, CHUNK], BF16, name=f"ex{p}{e}",
                                tag=f"spe{e}")
                nc.scalar.activation(out=ex[:], in_=selbank[p][e][:],
                                     func=AF.Exp, scale=-1.0)
                exs.append(ex)
            for e in range(2):
                # fp32 softplus values: the d subtraction cancels ~94% of
                # their magnitude, bf16 here costs a digit of accuracy
                sp_ = mpool.tile([128, CHUNK], F32, name=f"spl{p}{e}",
                                 tag=f"spl{e}")
                nc.scalar.activation(out=sp_[:], in_=exs[e][:],
                                     func=AF.Ln, bias=1.0)
                spl.append(sp_)
            if p == 0:
                # prefetch the Exp table set back in while pair 1's main
                # loop still runs, so pair 1's Exps start instantly
                nc.scalar.activation(out=tiny[:], in_=junkz[:, 0:1],
                                     func=AF.Exp)
            d = mpool.tile([128, CHUNK], BF16, tag="d")
            nc.vector.tensor_sub(d[:], spl[1][:], spl[0][:])
            junk = mpool.tile([128, CHUNK], BF16, tag="junk")
            nc.vector.scalar_tensor_tensor(
                out=junk[:], in0=d[:], scalar=1.0, in1=d[:],
                op0=ALU.mult, op1=ALU.mult,
                accum_out=acc_sb[:, p:p + 1])
        nc.sync.dma_start(acc[:], acc_sb[:])

    nc.compile()
    return nc


_prog = None


def _get_prog():
    global _prog
    if _prog is None:
        _prog = build()
    return _prog


def _l2norm_rows(x):
    n = np.sqrt((x.astype(np.float64) ** 2).sum(axis=1, keepdims=True))
    return (x / np.maximum(n, 1e-12)).astype(np.float32)


def _prep_inputs(z1, z2, g1, g2, batch_1, batch_2):
    import ml_dtypes
    z_np = (np.dtype("float32") if Z_DT == F32 else
            np.dtype("bfloat16") if Z_DT == BF16 else
            np.dtype(ml_dtypes.float8_e4m3fn))

    zn = [_l2norm_rows(np.asarray(z, np.float32)) for z in (z1, z2)]
    gn = [_l2norm_rows(np.asarray(g, np.float32)) for g in (g1, g2)]
    bs = [np.asarray(b).astype(np.int64).ravel() for b in (batch_1, batch_2)]

    # sel stationaries: e=0 selects s rows (w<16), e=1 selects c rows
    # (w>=16); out row j = chunk j-in-group, cols 4:32 unused (zero)
    sel = np.zeros((128, 2, 2 * WIN), np.float32)
    for j in range(4):
        sel[32 * j:32 * j + WIN, 0, j] = 1.0
        sel[32 * j + WIN:32 * j + 2 * WIN, 1, j] = 1.0
    sel = sel.astype(np.dtype("bfloat16"))

    in_maps = []
    for core in range(NCORES):
        sl = slice(core * RPC, (core + 1) * RPC)
        im = {"sel": sel}
        stc = np.zeros((128, 2, 16, 2, 2 * WIN), np.float32)
        for p in range(2):
            z = np.zeros((NODES, D), np.float32)
            z[:RPC] = zn[p][sl]
            zt = np.ascontiguousarray(
                z.T.reshape(2, 128, NODES).transpose(1, 0, 2))
            im[f"zT{p + 1}"] = zt.astype(z_np)

            b = bs[p][sl]
            g_self, g_cross = (gn[0], gn[1]) if p == 0 else (gn[1], gn[0])
            oh = np.zeros((128, NGRP, CHUNK), np.float32)
            for c in range(NCH):
                cb = b[c * CHUNK:min((c + 1) * CHUNK, RPC)]
                v0 = int(cb.min())
                span = int(cb.max()) - v0 + 1
                assert span <= WIN, f"core {core} pair {p} chunk {c}: " \
                    f"span {span} > {WIN}"
                nrows = min(WIN, G - v0)
                stc[:, p, c, :, 0:nrows] = \
                    g_self[v0:v0 + nrows].T.reshape(2, 128, nrows) \
                    .transpose(1, 0, 2)
                stc[:, p, c, :, WIN:WIN + nrows] = \
                    g_cross[v0:v0 + nrows].T.reshape(2, 128, nrows) \
                    .transpose(1, 0, 2)
                g_, j = divmod(c, 4)
                w = cb - v0
                i = np.arange(len(cb))
                oh[32 * j + w, g_, i] = 1.0
                oh[32 * j + WIN + w, g_, i] = 1.0
            im[f"oh{p + 1}"] = oh.astype(z_np)
        im["st"] = np.ascontiguousarray(stc).astype(z_np)
        in_maps.append(im)
    return in_maps


def _finish(results):
    tot = np.zeros(2, np.float64)
    for r in results:
        tot += r["acc"].astype(np.float64).sum(axis=0)
    return np.float32(np.sqrt(tot[0]) + np.sqrt(tot[1]))


def kernel(z1, z2, g1, g2, batch_1, batch_2, trace=False):
    nc = _get_prog()
    in_maps = _prep_inputs(z1, z2, g1, g2, batch_1, batch_2)
    res = run_bass_kernel_spmd(nc, in_maps, core_ids=list(range(NCORES)),
                               trace=trace)
    out = _finish(res.results)
    if trace:
        kernel.last_results = res
    return out


# revision 22
# speedup vs baseline: 1.1342x; 1.1342x over previous
"""Bass/Trainium2 kernel for nn_GCL_49959059587771 (GCL JSD loss).

Math: for pair (z, g, batch), with zn/gn row-normalized:
    s_i = <zn_i, gn_self[b_i]>,  c_i = <zn_i, gn_cross[b_i]>
    d_i = softplus(-c_i) - softplus(-s_i)
    L = sqrt(sum d1^2) + sqrt(sum d2^2)
(the one-hot mask collapses each [N, G] row to one entry; ep_jsd(0) = 0.)

Device strategy (8 cores, nodes sharded, v2 "tiny-window P" scheme):
  Host normalizes z and g rows (pure preprocessing, like the one-hot build)
  and ships per core:
    - zT      [128, 2, 6656] fp8: d-major transposed normalized z chunks
    - st      [128, 2, 16, 2, 32] fp8: per-(pair, chunk) stationary of 16
              self + 16 cross normalized-g window rows (batch is sorted, so
              each 512-node chunk spans < 16 distinct g rows)
    - oh      [128, 4, 512] bf16 per pair: one-hot window masks, 4 chunks
              stacked per 32-partition block
    - sel     [128, 32] bf16: partition-reduce stationary (s rows 0:4,
              c rows 8:12 per 32-block)
  Per (pair, group of 4 chunks): 8 tiny-stationary matmuls produce
  P[32*j + w, i] = <zn_i, gwin_w> in one PSUM bank; one DVE mask op and one
  select matmul reduce it to per-chunk s/c rows packed in a per-pair
  "select bank"; one ACT Softplus(-x) + 4 DVE subs + 1 DVE square-accум
  produce the per-pair sum(d^2) partials. Host sums partials, sqrt, add.
"""

import numpy as np
from contextlib import ExitStack

import concourse.bass as bass
import concourse.bacc as bacc
import concourse.tile as tile
import concourse.mybir as mybir
from concourse.bass_utils import run_bass_kernel_spmd

N, G, D = 50000, 512, 256
NCORES = 8
RPC = N // NCORES            # 6250 nodes per core
CHUNK = 512                  # nodes per chunk
NCH = 13                     # real chunks per core (13*512 = 6656 >= 6250)
NODES = NCH * CHUNK          # 6656 padded nodes per core
NGRP = 4                     # groups of 4 chunks (last group: 1 real chunk)
WIN = 16                     # g-row window per chunk (span asserted < 16)

AF = mybir.ActivationFunctionType
ALU = mybir.AluOpType
F32 = mybir.dt.float32
BF16 = mybir.dt.bfloat16
FP8 = mybir.dt.float8e4

Z_DT = FP8                   # dtype of zT and st (PE operands)


def build(z_dt=Z_DT, debug=False):
    nc = bacc.Bacc("TRN2", target_bir_lowering=False, debug=debug)

    zT1 = nc.dram_tensor("zT1", [128, 2, NODES], z_dt, kind="ExternalInput")
    zT2 = nc.dram_tensor("zT2", [128, 2, NODES], z_dt, kind="ExternalInput")
    oh1 = nc.dram_tensor("oh1", [128, NGRP, CHUNK], z_dt,
                         kind="ExternalInput")
    oh2 = nc.dram_tensor("oh2", [128, NGRP, CHUNK], z_dt,
                         kind="ExternalInput")
    st = nc.dram_tensor("st", [128, 2, 16, 2, 2 * WIN], z_dt,
                        kind="ExternalInput")
    sel = nc.dram_tensor("sel", [128, 2, 2 * WIN], BF16,
                         kind="ExternalInput")
    acc = nc.dram_tensor("acc", [128, 2], F32, kind="ExternalOutput")

    with tile.TileContext(nc) as tc, ExitStack() as ctx:
        singles = ctx.enter_context(tc.tile_pool(name="singles", bufs=1))
        zpool = ctx.enter_context(tc.tile_pool(name="z", bufs=3))
        mpool = ctx.enter_context(tc.tile_pool(name="m", bufs=3))
        ppool = ctx.enter_context(tc.tile_pool(name="pp", bufs=3,
                                               space="PSUM"))
        spool = ctx.enter_context(tc.tile_pool(name="sp", bufs=1,
                                               space="PSUM"))

        # all input DMAs go down ONE ring (a single ring sustains ~300GB/s
        # here; multiple rings just make the critical transfer compete for
        # HBM bandwidth), ordered so the pipeline's next need lands next
        st_sb = singles.tile([128, 2, 16, 2, 2 * WIN], z_dt)
        sel_sb = singles.tile([128, 2, 2 * WIN], BF16)
        oh_sb = [singles.tile([128, NGRP, CHUNK], z_dt, name=f"oh_sb{p}",
                              tag=f"oh{p}")
                 for p in range(2)]
        zg = [[None] * NGRP for _ in range(2)]
        for p, zT in enumerate((zT1, zT2)):
            for g in range(NGRP):
                lo = g * 4 * CHUNK
                hi = lo + min(4, NCH - 4 * g) * CHUNK
                zg[p][g] = zpool.tile([128, 2, hi - lo], z_dt,
                                      name=f"zg{p}{g}", tag=f"z{p}{g}")

        nc.sync.dma_start(st_sb[:], st[:])
        order = [("z", 0, 0), ("oh", 0, None), ("z", 0, 1), ("z", 0, 2),
                 ("sel", None, None), ("z", 0, 3), ("z", 1, 0),
                 ("oh", 1, None), ("z", 1, 1), ("z", 1, 2), ("z", 1, 3)]
        for kind, p, g in order:
            if kind == "z":
                lo = g * 4 * CHUNK
                hi = lo + min(4, NCH - 4 * g) * CHUNK
                zT = zT1 if p == 0 else zT2
                nc.sync.dma_start(zg[p][g][:], zT[:, :, lo:hi])
            elif kind == "oh":
                nc.sync.dma_start(oh_sb[p][:], (oh1, oh2)[p][:])
            else:
                nc.sync.dma_start(sel_sb[:], sel[:])

        # warm-up matmuls on a zeroed tile keep the PE HAM busy while the
        # first z transfer is in flight (otherwise every burst runs cold)
        junkz = singles.tile([128, CHUNK], BF16)
        nc.vector.memset(junkz[:], 0.0)
        dummy_ps = ctx.enter_context(tc.tile_pool(name="dps", bufs=1,
                                                  space="PSUM"))
        dps = dummy_ps.tile([128, CHUNK], F32)
        for _ in range(8):
            nc.tensor.matmul(dps[:], junkz[:, 0:128], junkz[:],
                             start=True, stop=True)

        # selbank[p][e]: e=0 holds s rows, e=1 holds c rows (chunk 4g+j of
        # pair p at partition 32g + j, only rows 32g + 0:4 of each block used)
        selbank = [[spool.tile([128, CHUNK], F32, name=f"selbank{p}{e}",
                               tag=f"sb{p}{e}")
                    for e in range(2)] for p in range(2)]

        # software-pipelined main loop: P-matmuls run one group ahead of
        # the dependent (DVE-gated) select matmuls so PE never stalls.
        work = [(p, g) for p in range(2) for g in range(NGRP)]
        pbank = {}

        def p_mms(p, g):
            nj = min(4, NCH - 4 * g)
            P = ppool.tile([128, CHUNK], F32, tag="P")
            for j in range(nj):
                c = 4 * g + j
                for k in range(2):
                    nc.tensor.matmul(
                        P[32 * j:32 * j + 32, :],
                        st_sb[:, p, c, k, :],
                        zg[p][g][:, k, (c - 4 * g) * CHUNK:
                                 (c - 4 * g + 1) * CHUNK],
                        start=(k == 0), stop=(k == 1),
                        tile_position=(0, 32 * j))
            pbank[(p, g)] = (P, nj)

        def mask_select(p, g):
            P, nj = pbank.pop((p, g))
            np_ = 32 * nj
            masked = mpool.tile([128, CHUNK], BF16, tag="masked")
            nc.vector.scalar_tensor_tensor(
                out=masked[0:np_, :], in0=oh_sb[p][0:np_, g, :], scalar=1.0,
                in1=P[0:np_, :], op0=ALU.mult, op1=ALU.mult)
            for e in range(2):
                nc.tensor.matmul(
                    selbank[p][e][32 * g:32 * g + 32, :],
                    sel_sb[0:np_, e, :], masked[0:np_, :],
                    start=True, stop=True, tile_position=(0, 32 * g))

        p_mms(*work[0])
        for i, (p, g) in enumerate(work):
            if i + 1 < len(work):
                p_mms(*work[i + 1])
            mask_select(p, g)

        # epilogue: softplus(-x) = ln(1 + exp(-x)) (Exp and Ln share one
        # table set); d = sp(-c) - sp(-s); accumulate d^2. Unused rows hold
        # selbank 0 -> sp ln2 on both sides -> d = 0.
        acc_sb = singles.tile([128, 2], F32)
        tiny = singles.tile([128, 1], F32)
        for p in range(2):
            spl = []
            exs = []
            for e in range(2):
                ex = mpool.tile([128, CHUNK], BF16, name=f"ex{p}{e}",
                                tag=f"spe{e}")
                nc.scalar.activation(out=ex[:], in_=selbank[p][e][:],
                                     func=AF.Exp, scale=-1.0)
                exs.append(ex)
            for e in range(2):
                # fp32 softplus values: the d subtraction cancels ~94% of
                # their magnitude, bf16 here costs a digit of accuracy
                sp_ = mpool.tile([128, CHUNK], F32, name=f"spl{p}{e}",
                                 tag=f"spl{e}")
                nc.scalar.activation(out=sp_[:], in_=exs[e][:],
                                     func=AF.Ln, bias=1.0)
                spl.append(sp_)
            if p == 0:
                # prefetch the Exp table set back in while pair 1's main
                # loop still runs, so pair 1's Exps start instantly
                nc.scalar.activation(out=tiny[:], in_=junkz[:, 0:1],
                                     func=AF.Exp)
            d = mpool.tile([128, CHUNK], BF16, tag="d")
            nc.vector.tensor_sub(d[:], spl[1][:], spl[0][:])
            junk = mpool.tile([128, CHUNK], BF16, tag="junk")
            nc.vector.scalar_tensor_tensor(
                out=junk[:], in0=d[:], scalar=1.0, in1=d[:],
                op0=ALU.mult, op1=ALU.mult,
                accum_out=acc_sb[:, p:p + 1])
        nc.sync.dma_start(acc[:], acc_sb[:])

    nc.compile()
    return nc


_prog = None


def _get_prog():
    global _prog
    if _prog is None:
        _prog = build()
    return _prog


def _l2norm_rows(x):
    n = np.sqrt((x.astype(np.float64) ** 2).sum(axis=1, keepdims=True))
    return (x / np.maximum(n, 1e-12)).astype(np.float32)


def _prep_inputs(z1, z2, g1, g2, batch_1, batch_2):
    import ml_dtypes
    z_np = (np.dtype("float32") if Z_DT == F32 else
            np.dtype("bfloat16") if Z_DT == BF16 else
            np.dtype(ml_dtypes.float8_e4m3fn))

    zn = [_l2norm_rows(np.asarray(z, np.float32)) for z in (z1, z2)]
    gn = [_l2norm_rows(np.asarray(g, np.float32)) for g in (g1, g2)]
    bs = [np.asarray(b).astype(np.int64).ravel() for b in (batch_1, batch_2)]

    # sel stationaries: e=0 selects s rows (w<16), e=1 selects c rows
    # (w>=16); out row j = chunk j-in-group, cols 4:32 unused (zero)
    sel = np.zeros((128, 2, 2 * WIN), np.float32)
    for j in range(4):
        sel[32 * j:32 * j + WIN, 0, j] = 1.0
        sel[32 * j + WIN:32 * j + 2 * WIN, 1, j] = 1.0
    sel = sel.astype(np.dtype("bfloat16"))

    in_maps = []
    for core in range(NCORES):
        sl = slice(core * RPC, (core + 1) * RPC)
        im = {"sel": sel}
        stc = np.zeros((128, 2, 16, 2, 2 * WIN), np.float32)
        for p in range(2):
            z = np.zeros((NODES, D), np.float32)
            z[:RPC] = zn[p][sl]
            zt = np.ascontiguousarray(
                z.T.reshape(2, 128, NODES).transpose(1, 0, 2))
            im[f"zT{p + 1}"] = zt.astype(z_np)

            b = bs[p][sl]
            g_self, g_cross = (gn[0], gn[1]) if p == 0 else (gn[1], gn[0])
            oh = np.zeros((128, NGRP, CHUNK), np.float32)
            for c in range(NCH):
                cb = b[c * CHUNK:min((c + 1) * CHUNK, RPC)]
                v0 = int(cb.min())
                span = int(cb.max()) - v0 + 1
                assert span <= WIN, f"core {core} pair {p} chunk {c}: " \
                    f"span {span} > {WIN}"
                nrows = min(WIN, G - v0)
                stc[:, p, c, :, 0:nrows] = \
                    g_self[v0:v0 + nrows].T.reshape(2, 128, nrows) \
                    .transpose(1, 0, 2)
                stc[:, p, c, :, WIN:WIN + nrows] = \
                    g_cross[v0:v0 + nrows].T.reshape(2, 128, nrows) \
                    .transpose(1, 0, 2)
                g_, j = divmod(c, 4)
                w = cb - v0
                i = np.arange(len(cb))
                oh[32 * j + w, g_, i] = 1.0
                oh[32 * j + WIN + w, g_, i] = 1.0
            im[f"oh{p + 1}"] = oh.astype(z_np)
        im["st"] = np.ascontiguousarray(stc).astype(z_np)
        in_maps.append(im)
    return in_maps


def _finish(results):
    tot = np.zeros(2, np.float64)
    for r in results:
        tot += r["acc"].astype(np.float64).sum(axis=0)
    return np.float32(np.sqrt(tot[0]) + np.sqrt(tot[1]))


def kernel(z1, z2, g1, g2, batch_1, batch_2, trace=False):
    nc = _get_prog()
    in_maps = _prep_inputs(z1, z2, g1, g2, batch_1, batch_2)
    res = run_bass_kernel_spmd(nc, in_maps, core_ids=list(range(NCORES)),
                               trace=trace)
    out = _finish(res.results)
    if trace:
        kernel.last_results = res
    return out


# revision 29
# speedup vs baseline: 1.2668x; 1.1169x over previous
"""Bass/Trainium2 kernel for nn_GCL_49959059587771 (GCL JSD loss).

Math: for pair (z, g, batch), with zn/gn row-normalized:
    s_i = <zn_i, gn_self[b_i]>,  c_i = <zn_i, gn_cross[b_i]>
    d_i = softplus(-c_i) - softplus(-s_i)
    L = sqrt(sum d1^2) + sqrt(sum d2^2)
(the one-hot mask collapses each [N, G] row to one entry; ep_jsd(0) = 0.)

Device strategy (8 cores, nodes sharded, v2 "tiny-window P" scheme):
  Host normalizes z and g rows (pure preprocessing, like the one-hot build)
  and ships per core:
    - zT      [128, 2, 6656] fp8: d-major transposed normalized z chunks
    - st      [128, 2, 16, 2, 32] fp8: per-(pair, chunk) stationary of 16
              self + 16 cross normalized-g window rows (batch is sorted, so
              each 512-node chunk spans < 16 distinct g rows)
    - oh      [128, 4, 512] bf16 per pair: one-hot window masks, 4 chunks
              stacked per 32-partition block
    - sel     [128, 32] bf16: partition-reduce stationary (s rows 0:4,
              c rows 8:12 per 32-block)
  Per (pair, group of 4 chunks): 8 tiny-stationary matmuls produce
  P[32*j + w, i] = <zn_i, gwin_w> in one PSUM bank; one DVE mask op and one
  select matmul reduce it to per-chunk s/c rows packed in a per-pair
  "select bank"; one ACT Softplus(-x) + 4 DVE subs + 1 DVE square-accум
  produce the per-pair sum(d^2) partials. Host sums partials, sqrt, add.
"""

import numpy as np
from contextlib import ExitStack

import concourse.bass as bass
import concourse.bacc as bacc
import concourse.tile as tile
import concourse.mybir as mybir
from concourse.bass_utils import run_bass_kernel_spmd

N, G, D = 50000, 512, 256
NCORES = 8
RPC = N // NCORES            # 6250 nodes per core
CHUNK = 512                  # nodes per chunk
NCH = 13                     # real chunks per core (13*512 = 6656 >= 6250)
NODES = NCH * CHUNK          # 6656 padded nodes per core
NGRP = 4                     # groups of 4 chunks (last group: 1 real chunk)
WIN = 16                     # g-row window per chunk (span asserted < 16)

AF = mybir.ActivationFunctionType
ALU = mybir.AluOpType
F32 = mybir.dt.float32
BF16 = mybir.dt.bfloat16
FP8 = mybir.dt.float8e4

Z_DT = FP8                   # dtype of zT and st (PE operands)


def _pinned_act_tables(arch):
    """Only offer table sets containing BOTH Exp and Ln, so every
    activation in this kernel shares one resident set (a single
    ACT_TABLE_LOAD instead of Exp<->Ln thrash at ~1.3us per switch)."""
    import concourse.hw_specs as hw_specs
    tabs = hw_specs.get_activation_tables(arch)
    need = {AF.Exp, AF.Ln}
    if not any(need <= v for v in tabs.values()):
        return tabs
    # keep names/indices intact (set id = position in this dict); just make
    # the unwanted sets unselectable
    return {k: (v if need <= v else set()) for k, v in tabs.items()}


def build(z_dt=Z_DT, debug=False):
    orig_tables = bacc.get_activation_tables
    bacc.get_activation_tables = _pinned_act_tables
    try:
        return _build(z_dt, debug)
    finally:
        bacc.get_activation_tables = orig_tables


def _build(z_dt, debug):
    nc = bacc.Bacc("TRN2", target_bir_lowering=False, debug=debug)

    zT1 = nc.dram_tensor("zT1", [128, 2, NODES], z_dt, kind="ExternalInput")
    zT2 = nc.dram_tensor("zT2", [128, 2, NODES], z_dt, kind="ExternalInput")
    oh1 = nc.dram_tensor("oh1", [128, NGRP, CHUNK], z_dt,
                         kind="ExternalInput")
    oh2 = nc.dram_tensor("oh2", [128, NGRP, CHUNK], z_dt,
                         kind="ExternalInput")
    st = nc.dram_tensor("st", [128, 2, 16, 2, 2 * WIN], z_dt,
                        kind="ExternalInput")
    sel = nc.dram_tensor("sel", [128, 2, 2 * WIN], BF16,
                         kind="ExternalInput")
    acc = nc.dram_tensor("acc", [128, 2], F32, kind="ExternalOutput")

    with tile.TileContext(nc) as tc, ExitStack() as ctx:
        singles = ctx.enter_context(tc.tile_pool(name="singles", bufs=1))
        zpool = ctx.enter_context(tc.tile_pool(name="z", bufs=1))
        mpool = ctx.enter_context(tc.tile_pool(name="m", bufs=3))
        ppool = ctx.enter_context(tc.tile_pool(name="pp", bufs=3,
                                               space="PSUM"))
        spool = ctx.enter_context(tc.tile_pool(name="sp", bufs=1,
                                               space="PSUM"))

        # all input DMAs go down ONE ring (a single ring sustains ~300GB/s
        # here; multiple rings just make the critical transfer compete for
        # HBM bandwidth), ordered so the pipeline's next need lands next
        st_sb = singles.tile([128, 2, 16, 2, 2 * WIN], z_dt)
        sel_sb = singles.tile([128, 2, 2 * WIN], BF16)
        oh_sb = [singles.tile([128, NGRP, CHUNK], z_dt, name=f"oh_sb{p}",
                              tag=f"oh{p}")
                 for p in range(2)]
        # z tiles: group 0 alone (unblocks the first matmuls ASAP), groups
        # 1-3 as one transfer per pair (fewer issue slots on the ring)
        REST = (NCH - 4) * CHUNK
        z0 = [zpool.tile([128, 2, 4 * CHUNK], z_dt, name=f"z0_{p}",
                         tag=f"z0{p}") for p in range(2)]
        zrest = [zpool.tile([128, 2, REST], z_dt, name=f"zrest{p}",
                            tag=f"zr{p}") for p in range(2)]

        def zslice(p, c, k):
            if c < 4:
                return z0[p][:, k, c * CHUNK:(c + 1) * CHUNK]
            return zrest[p][:, k, (c - 4) * CHUNK:(c - 3) * CHUNK]

        nc.sync.dma_start(st_sb[:], st[:])
        nc.sync.dma_start(z0[0][:], zT1[:, :, 0:4 * CHUNK])
        nc.sync.dma_start(oh_sb[0][:], oh1[:])
        nc.sync.dma_start(sel_sb[:], sel[:])
        nc.sync.dma_start(zrest[0][:], zT1[:, :, 4 * CHUNK:NODES])
        nc.sync.dma_start(z0[1][:], zT2[:, :, 0:4 * CHUNK])
        nc.sync.dma_start(oh_sb[1][:], oh2[:])
        nc.sync.dma_start(zrest[1][:], zT2[:, :, 4 * CHUNK:NODES])

        # warm-up matmuls on a zeroed tile keep the PE HAM busy while the
        # first z transfer is in flight (otherwise every burst runs cold)
        junkz = singles.tile([128, CHUNK], BF16)
        nc.vector.memset(junkz[:], 0.0)
        dummy_ps = ctx.enter_context(tc.tile_pool(name="dps", bufs=1,
                                                  space="PSUM"))
        dps = dummy_ps.tile([128, CHUNK], F32)
        for _ in range(8):
            nc.tensor.matmul(dps[:], junkz[:, 0:128], junkz[:],
                             start=True, stop=True)

        # selbank[p][e]: e=0 holds s rows, e=1 holds c rows (chunk 4g+j of
        # pair p at partition 32g + j, only rows 32g + 0:4 of each block used)
        selbank = [[spool.tile([128, CHUNK], F32, name=f"selbank{p}{e}",
                               tag=f"sb{p}{e}")
                    for e in range(2)] for p in range(2)]

        # software-pipelined main loop: P-matmuls run one group ahead of
        # the dependent (DVE-gated) select matmuls so PE never stalls.
        work = [(p, g) for p in range(2) for g in range(NGRP)]
        pbank = {}

        def p_mms(p, g):
            nj = min(4, NCH - 4 * g)
            P = ppool.tile([128, CHUNK], F32, tag="P")
            for j in range(nj):
                c = 4 * g + j
                for k in range(2):
                    nc.tensor.matmul(
                        P[32 * j:32 * j + 32, :],
                        st_sb[:, p, c, k, :], zslice(p, c, k),
                        start=(k == 0), stop=(k == 1),
                        tile_position=(0, 32 * j))
            pbank[(p, g)] = (P, nj)

        def mask_select(p, g):
            P, nj = pbank.pop((p, g))
            np_ = 32 * nj
            masked = mpool.tile([128, CHUNK], BF16, tag="masked")
            nc.vector.scalar_tensor_tensor(
                out=masked[0:np_, :], in0=oh_sb[p][0:np_, g, :], scalar=1.0,
                in1=P[0:np_, :], op0=ALU.mult, op1=ALU.mult)
            for e in range(2):
                nc.tensor.matmul(
                    selbank[p][e][32 * g:32 * g + 32, :],
                    sel_sb[0:np_, e, :], masked[0:np_, :],
                    start=True, stop=True, tile_position=(0, 32 * g))

        p_mms(*work[0])
        for i, (p, g) in enumerate(work):
            if i + 1 < len(work):
                p_mms(*work[i + 1])
            mask_select(p, g)

        # epilogue: softplus(-x) = ln(1 + exp(-x)) (Exp and Ln share one
        # table set); d = sp(-c) - sp(-s); accumulate d^2. Unused rows hold
        # selbank 0 -> sp ln2 on both sides -> d = 0.
        acc_sb = singles.tile([128, 2], F32)
        for p in range(2):
            spl = []
            exs = []
            for e in range(2):
                ex = mpool.tile([128, CHUNK], BF16, name=f"ex{p}{e}",
                                tag=f"spe{e}")
                nc.scalar.activation(out=ex[:], in_=selbank[p][e][:],
                                     func=AF.Exp, scale=-1.0)
                exs.append(ex)
            for e in range(2):
                # fp32 softplus values: the d subtraction cancels ~94% of
                # their magnitude, bf16 here costs a digit of accuracy
                sp_ = mpool.tile([128, CHUNK], F32, name=f"spl{p}{e}",
                                 tag=f"spl{e}")
                nc.scalar.activation(out=sp_[:], in_=exs[e][:],
                                     func=AF.Ln, bias=1.0)
                spl.append(sp_)
            d = mpool.tile([128, CHUNK], BF16, tag="d")
            nc.vector.tensor_sub(d[:], spl[1][:], spl[0][:])
            junk = mpool.tile([128, CHUNK], BF16, tag="junk")
            nc.vector.scalar_tensor_tensor(
                out=junk[:], in0=d[:], scalar=1.0, in1=d[:],
                op0=ALU.mult, op1=ALU.mult,
                accum_out=acc_sb[:, p:p + 1])
        nc.sync.dma_start(acc[:], acc_sb[:])

    nc.compile()
    return nc


_prog = None


def _get_prog():
    global _prog
    if _prog is None:
        _prog = build()
    return _prog


def _l2norm_rows(x):
    n = np.sqrt((x.astype(np.float64) ** 2).sum(axis=1, keepdims=True))
    return (x / np.maximum(n, 1e-12)).astype(np.float32)


def _prep_inputs(z1, z2, g1, g2, batch_1, batch_2):
    import ml_dtypes
    z_np = (np.dtype("float32") if Z_DT == F32 else
            np.dtype("bfloat16") if Z_DT == BF16 else
            np.dtype(ml_dtypes.float8_e4m3fn))

    zn = [_l2norm_rows(np.asarray(z, np.float32)) for z in (z1, z2)]
    gn = [_l2norm_rows(np.asarray(g, np.float32)) for g in (g1, g2)]
    bs = [np.asarray(b).astype(np.int64).ravel() for b in (batch_1, batch_2)]

    # sel stationaries: e=0 selects s rows (w<16), e=1 selects c rows
    # (w>=16); out row j = chunk j-in-group, cols 4:32 unused (zero)
    sel = np.zeros((128, 2, 2 * WIN), np.float32)
    for j in range(4):
        sel[32 * j:32 * j + WIN, 0, j] = 1.0
        sel[32 * j + WIN:32 * j + 2 * WIN, 1, j] = 1.0
    sel = sel.astype(np.dtype("bfloat16"))

    in_maps = []
    for core in range(NCORES):
        sl = slice(core * RPC, (core + 1) * RPC)
        im = {"sel": sel}
        stc = np.zeros((128, 2, 16, 2, 2 * WIN), np.float32)
        for p in range(2):
            z = np.zeros((NODES, D), np.float32)
            z[:RPC] = zn[p][sl]
            zt = np.ascontiguousarray(
                z.T.reshape(2, 128, NODES).transpose(1, 0, 2))
            im[f"zT{p + 1}"] = zt.astype(z_np)

            b = bs[p][sl]
            g_self, g_cross = (gn[0], gn[1]) if p == 0 else (gn[1], gn[0])
            oh = np.zeros((128, NGRP, CHUNK), np.float32)
            for c in range(NCH):
                cb = b[c * CHUNK:min((c + 1) * CHUNK, RPC)]
                v0 = int(cb.min())
                span = int(cb.max()) - v0 + 1
                assert span <= WIN, f"core {core} pair {p} chunk {c}: " \
                    f"span {span} > {WIN}"
                nrows = min(WIN, G - v0)
                stc[:, p, c, :, 0:nrows] = \
                    g_self[v0:v0 + nrows].T.reshape(2, 128, nrows) \
                    .transpose(1, 0, 2)
                stc[:, p, c, :, WIN:WIN + nrows] = \
                    g_cross[v0:v0 + nrows].T.reshape(2, 128, nrows) \
                    .transpose(1, 0, 2)
                g_, j = divmod(c, 4)
                w = cb - v0
                i = np.arange(len(cb))
                oh[32 * j + w, g_, i] = 1.0
                oh[32 * j + WIN + w, g_, i] = 1.0
            im[f"oh{p + 1}"] = oh.astype(z_np)
        im["st"] = np.ascontiguousarray(stc).astype(z_np)
        in_maps.append(im)
    return in_maps


def _finish(results):
    tot = np.zeros(2, np.float64)
    for r in results:
        tot += r["acc"].astype(np.float64).sum(axis=0)
    return np.float32(np.sqrt(tot[0]) + np.sqrt(tot[1]))


def kernel(z1, z2, g1, g2, batch_1, batch_2, trace=False):
    nc = _get_prog()
    in_maps = _prep_inputs(z1, z2, g1, g2, batch_1, batch_2)
    res = run_bass_kernel_spmd(nc, in_maps, core_ids=list(range(NCORES)),
                               trace=trace)
    out = _finish(res.results)
    if trace:
        kernel.last_results = res
    return out
